# revision 22
# baseline (speedup 1.0000x reference)
"""MoE layer (shared expert + 8 routed experts, top-2 sigmoid router) on 8
Trainium2 NeuronCores — sparse dispatch version.

Strategy: data-parallel over tokens (1024/core). Each core:
  1. Router in exact fp32 on PE (top-2 via DVE max8 + match_replace).
  2. Sparse dispatch: each token goes to only its top-2 experts via SWDGE
     dma_gather into a per-expert capacity buffer (CAP=384, E[count]=256).
     The gate weight tw is folded in as x*sqrt(tw) before dispatch since
     relu(a*x@w1)^2 @ w2 = a^2 * (relu(x@w1)^2 @ w2).
  3. The slot->token map is built ON DEVICE: exclusive cumsum over the
     top-2 masks (triangular-matrix matmuls), inverted by scatter-adding
     per-token metadata rows into a slot-indexed DRAM array, read back
     directly in the SWDGE index layout via a strided DMA.
  4. Routed experts run in bf16; layer 2 is slot-major (stationary = h^T)
     so outputs are slot rows, scatter-added onto the token-major output
     pre-filled with the shared-expert MLP.

Emission order is tuned so the dispatch-index critical path (DVE + gpsimd)
completes while the PE runs the shared expert, and small index DMAs are
spread across the scalar/vector/gpsimd queues to avoid head-of-line
blocking behind bulk weight DMAs on the sync queue.
"""
import sys

sys.path.insert(0, '/opt/trn_rl_repo')

import numpy as np
import ml_dtypes

import concourse.bass as bass
import concourse.mybir as mybir
import concourse.tile as tile
from concourse import bacc
from concourse.bass_utils import run_bass_kernel_spmd
from concourse.masks import make_identity

f32 = mybir.dt.float32
bf16 = mybir.dt.bfloat16
i16 = mybir.dt.int16
AF = mybir.ActivationFunctionType
ALU = mybir.AluOpType

N_CORES = 8
B, T, C = 4, 2048, 768
E, K = 8, 2
N_TOK = B * T
TLOC = N_TOK // N_CORES      # tokens per core (1024)
KT = C // 128                # 6 contraction tiles
TB = TLOC // 128             # 8 token blocks
CAP = 384                    # per-expert slot capacity (mean count = 256)
S = E * CAP                  # 3072 total slots
SB = S // 128                # 24 slot blocks
ECOLS = CAP // 16            # idx columns per expert (24)
EB = CAP // 128              # slot blocks per expert (3)
XS_ROWS = 2 * TLOC + 128     # dispatch buffer rows (rows 2048.. = zeros)
ZROW = 2 * TLOC              # zero row index
OUT_ROWS = TLOC + 128        # output rows (row 1024 = pad-slot dummy)
DUMMY = TLOC


def _build():
    nc = bacc.Bacc("TRN2", target_bir_lowering=False, debug=False,
                   num_devices=N_CORES)

    x_T = nc.declare_dram_parameter("x_T", [C, TLOC], f32, isOutput=False)
    x_T16 = nc.declare_dram_parameter("x_T16", [C, TLOC], bf16, isOutput=False)
    x_tm = nc.declare_dram_parameter("x_tm", [TLOC, C], f32, isOutput=False)
    rwT = nc.declare_dram_parameter("rwT", [C, E], f32, isOutput=False)
    w1 = nc.declare_dram_parameter("w1", [E, C, C], bf16, isOutput=False)
    w2 = nc.declare_dram_parameter("w2", [E, C, C], bf16, isOutput=False)
    wfc = nc.declare_dram_parameter("wfc", [C, C], bf16, isOutput=False)
    wproj = nc.declare_dram_parameter("wproj", [C, C], bf16, isOutput=False)
    trid = nc.declare_dram_parameter("trid", [128, 128], f32, isOutput=False)
    e384row = nc.declare_dram_parameter("e384row", [1, TB * E], f32,
                                        isOutput=False)
    brow = nc.declare_dram_parameter("brow", [1, TB * E], f32, isOutput=False)
    cbase = nc.declare_dram_parameter("cbase", [128, 2 * TB, 2], f32,
                                      isOutput=False)
    o_out = nc.declare_dram_parameter("o_out", [OUT_ROWS, C], f32,
                                      isOutput=True)
    o_dbg = nc.declare_dram_parameter("o_dbg", [128, 16], f32, isOutput=True)

    xs_dram = nc.dram_tensor("xs_dram", [XS_ROWS, C], bf16)
    meta_dram = nc.dram_tensor("meta_dram", [S, 64], f32)

    with tile.TileContext(nc) as tc:
        with (
            tc.tile_pool(name="const", bufs=1) as cpool,
            tc.tile_pool(name="acts", bufs=1) as apool,
            tc.tile_pool(name="wts", bufs=2) as wpool,
            tc.tile_pool(name="rt", bufs=2) as rpool,
            tc.tile_pool(name="masks", bufs=1) as mpool,
            tc.tile_pool(name="idx", bufs=1) as ipool,
            tc.tile_pool(name="xg", bufs=2) as gpool,
            tc.tile_pool(name="hsq", bufs=2) as hpool,
            tc.tile_pool(name="ysb", bufs=2) as ypool,
            tc.tile_pool(name="ysh", bufs=2) as yshpool,
            tc.tile_pool(name="xsc", bufs=2) as xspool,
            tc.tile_pool(name="ps_small", bufs=1, space="PSUM") as ps_s,
            tc.tile_pool(name="ps_t", bufs=1, space="PSUM") as ps_t,
            tc.tile_pool(name="ps_l1", bufs=2, space="PSUM") as ps_l1,
            tc.tile_pool(name="ps_l2", bufs=2, space="PSUM") as ps_l2,
        ):
            # ---------------- constants / inputs ----------------
            rwt = cpool.tile([128, KT, E], f32)
            nc.sync.dma_start(rwt[:], rwT.rearrange("(k p) e -> p k e", p=128))
            xt = []
            for k in range(KT):
                xt_k = apool.tile([128, TLOC], f32, tag=f"xt{k}")
                nc.sync.dma_start(xt_k[:], x_T[k * 128:(k + 1) * 128, :])
                xt.append(xt_k)
            tri_sb = cpool.tile([128, 128], f32)
            nc.sync.dma_start(tri_sb[:], trid[:])
            e384_sb = cpool.tile([128, TB, E], f32)
            nc.sync.dma_start(
                e384_sb[:].rearrange("p a b -> p (a b)"),
                e384row[0:1, :].to_broadcast([128, TB * E]))
            ones_sb = cpool.tile([128, 128], f32)
            nc.vector.memset(ones_sb[:], 1.0)
            ident = cpool.tile([128, 128], f32)
            make_identity(nc, ident[:])
            bias_sb = cpool.tile([128, TB, E], f32)
            nc.sync.dma_start(
                bias_sb[:].rearrange("p a b -> p (a b)"),
                brow[0:1, :].to_broadcast([128, TB * E]))

            # zero the slot-metadata array early (gpsimd queue)
            zmeta = ipool.tile([128, SB, 64], f32)
            nc.vector.memset(zmeta[:], 0.0)
            nc.gpsimd.dma_start(meta_dram.rearrange("(b p) c -> p b c", p=128),
                                zmeta[:])

            x16 = apool.tile([128, KT, TLOC], bf16)
            nc.sync.dma_start(x16[:], x_T16.rearrange("(k p) n -> p k n", p=128))
            xtm = apool.tile([128, TB, C], f32)
            nc.sync.dma_start(xtm[:], x_tm.rearrange("(b p) c -> p b c", p=128))

            wfc_sb = wpool.tile([128, KT, C], bf16, tag="w1")
            wproj_sb = wpool.tile([128, KT, C], bf16, tag="w2")
            for k in range(KT):
                nc.sync.dma_start(
                    wfc_sb[:, k, :],
                    wfc.rearrange("(k p) m -> p k m", p=128)[:, k, :])
                nc.sync.dma_start(
                    wproj_sb[:, k, :],
                    wproj.rearrange("(k p) m -> p k m", p=128)[:, k, :])

            slots_tm = ipool.tile([128, 2 * TB], f32)     # col b = k*8+tb
            content = ipool.tile([128, 2 * TB, 64], f32)  # scatter payload
            nc.vector.memset(content[:], 0.0)
            # payload values are routing-independent -> precomputed on host
            nc.sync.dma_start(content[:, :, 0:2], cbase[:])

            # ---------------- router (transposed matmuls, batched top-2) ---
            scoresT = ipool.tile([8, TLOC], f32)
            for half in range(2):
                hs = slice(half * 512, (half + 1) * 512)
                ps_rt = ps_t.tile([8, 512], f32, tag="t8")
                for k in range(KT):
                    nc.tensor.matmul(ps_rt[:], rwt[:, k, :], xt[k][:, hs],
                                     start=(k == 0), stop=(k == KT - 1))
                nc.scalar.activation(scoresT[:, hs], ps_rt[:], AF.Sigmoid)
            scores_all = ipool.tile([128, TB, E], f32)
            for tb in range(TB):
                blk = slice(tb * 128, (tb + 1) * 128)
                tp = ps_s.tile([128, E], f32, tag="small")
                nc.tensor.transpose(tp[:], scoresT[:, blk], ident[0:8, 0:8])
                nc.vector.tensor_copy(scores_all[:, tb, :], tp[:])
            sel_all = ipool.tile([128, TB, E], f32)
            nc.vector.tensor_add(sel_all[:], scores_all[:], bias_sb[:])
            top1 = ipool.tile([128, TB, 1], f32)
            nc.vector.reduce_max(top1[:, :, 0], sel_all[:],
                                 mybir.AxisListType.X)
            m1_all = ipool.tile([128, TB, E], f32)
            nc.vector.tensor_tensor(m1_all[:], sel_all[:],
                                    top1[:].to_broadcast((128, TB, E)),
                                    mybir.AluOpType.is_equal)
            sc2 = ipool.tile([128, TB, E], f32)
            nc.vector.tensor_mul(sc2[:], m1_all[:], sel_all[:])
            nc.vector.tensor_sub(sc2[:], sel_all[:], sc2[:])
            top2 = ipool.tile([128, TB, 1], f32)
            nc.vector.reduce_max(top2[:, :, 0], sc2[:], mybir.AxisListType.X)
            m2_all = ipool.tile([128, TB, E], f32)
            nc.vector.tensor_tensor(m2_all[:], sc2[:],
                                    top2[:].to_broadcast((128, TB, E)),
                                    mybir.AluOpType.is_equal)
            m_all = ipool.tile([128, TB, E], f32)
            nc.vector.tensor_add(m_all[:], m1_all[:], m2_all[:])
            # gate values come from the UNbiased scores at selected slots
            v_all = ipool.tile([128, TB, K], f32)
            tsel = ipool.tile([128, TB, E], f32)
            nc.vector.tensor_mul(tsel[:], m1_all[:], scores_all[:])
            nc.vector.reduce_sum(v_all[:, :, 0], tsel[:], mybir.AxisListType.X)
            nc.vector.tensor_mul(tsel[:], m2_all[:], scores_all[:])
            nc.vector.reduce_sum(v_all[:, :, 1], tsel[:], mybir.AxisListType.X)
            den = ipool.tile([128, TB], f32)
            nc.vector.tensor_add(den[:], v_all[:, :, 0], v_all[:, :, 1])
            rden = ipool.tile([128, TB], f32)
            nc.vector.reciprocal(rden[:], den[:])
            twv_all = ipool.tile([128, TB, K], f32)
            nc.vector.tensor_mul(twv_all[:, :, 0], v_all[:, :, 0], rden[:])
            nc.vector.tensor_mul(twv_all[:, :, 1], v_all[:, :, 1], rden[:])
            sqtw_all = ipool.tile([128, TB, K], f32)
            nc.scalar.activation(sqtw_all[:], twv_all[:], AF.Sqrt)

            # ---------------- positions via exclusive cumsum (transposed) --
            posT = ipool.tile([8, TLOC], f32)
            for tb in range(TB):
                blk = slice(tb * 128, (tb + 1) * 128)
                ps_ct = ps_t.tile([8, 512], f32, tag="t8")
                for tb2 in range(tb):
                    nc.tensor.matmul(ps_ct[:, 0:128], m_all[:, tb2, :],
                                     ones_sb[:],
                                     start=(tb2 == 0), stop=False)
                nc.tensor.matmul(ps_ct[:, 0:128], m_all[:, tb, :], tri_sb[:],
                                 start=(tb == 0), stop=True)
                nc.scalar.activation(posT[:, blk], ps_ct[:, 0:128], AF.Copy)
            sl_all = ipool.tile([128, TB, E], f32)
            for tb in range(TB):
                blk = slice(tb * 128, (tb + 1) * 128)
                tp = ps_s.tile([128, E], f32, tag="small")
                nc.tensor.transpose(tp[:], posT[:, blk], ident[0:8, 0:8])
                nc.vector.tensor_scalar_min(sl_all[:, tb, :], tp[:],
                                            float(CAP - 1))
            nc.vector.tensor_add(sl_all[:], sl_all[:], e384_sb[:])
            t1_all = ipool.tile([128, TB, E], f32)
            nc.vector.tensor_mul(t1_all[:], sl_all[:], m1_all[:])
            nc.vector.reduce_sum(slots_tm[:, 0:TB], t1_all[:],
                                 mybir.AxisListType.X)
            nc.vector.tensor_mul(t1_all[:], sl_all[:], m2_all[:])
            nc.vector.reduce_sum(slots_tm[:, TB:2 * TB], t1_all[:],
                                 mybir.AxisListType.X)

            # ---------------- invert the slot map via scatter-add ----------
            slots_i16 = ipool.tile([128, 2 * TB], i16)
            nc.vector.tensor_copy(slots_i16[:], slots_tm[:])
            # scaled dispatch copies (DVE; only needs sqtw) — must precede
            # the first gather but not the scatter/readback chain
            for tb in range(TB):
                for k in range(K):
                    xsk = xspool.tile([128, C], bf16, tag=f"xs{k}")
                    nc.vector.tensor_scalar_mul(xsk[:], xtm[:, tb, :],
                                                sqtw_all[:, tb, k:k + 1])
                    nc.sync.dma_start(
                        xs_dram[k * TLOC + tb * 128:k * TLOC + (tb + 1) * 128, :],
                        xsk[:])
            zrow16 = ipool.tile([128, C], bf16)
            nc.vector.memset(zrow16[:], 0.0)
            nc.sync.dma_start(xs_dram[ZROW:ZROW + 128, :], zrow16[:])
            nc.sync.dma_start(o_dbg[:], slots_tm[:])
            inv_idx = ipool.tile([128, 2 * TLOC // 16], i16)  # [128, 128]
            inv_r = inv_idx[:].rearrange("p (c r) -> p c r", r=8)
            for r in range(8):
                eng = nc.scalar if r % 2 == 0 else nc.gpsimd
                eng.dma_start(inv_r[0:16, :, r],
                              slots_i16[r * 16:(r + 1) * 16, :])
            for rr in range(1, 8):
                eng = nc.scalar if rr % 2 == 0 else nc.gpsimd
                eng.dma_start(inv_idx[rr * 16:(rr + 1) * 16, :],
                              inv_idx[0:16, :])

            nc.gpsimd.dma_scatter_add(meta_dram[:], content[:], inv_idx[:],
                                      2 * TLOC, 2 * TLOC, 64)
            # strided readback straight into the SWDGE idx wrap layout:
            # row (b*128 + r*16 + q) -> [q, b*8+r]. The same metadata is
            # read 8x in parallel (one DMA per 16-partition group) so the
            # idx tiles come out already replicated for the 8 gpsimd cores
            # and no serial replicate chain sits before the first gather.
            gsb = ipool.tile([128, S // 16, 2], f32)
            meta_r = meta_dram.rearrange("(b r q) c -> q (b r) c", q=16, r=8)
            for g in range(8):
                eng = nc.scalar if g % 2 == 0 else nc.gpsimd
                eng.dma_start(gsb[g * 16:(g + 1) * 16, :, :],
                              meta_r[:, :, 0:2])

            gidx_f = ipool.tile([128, S // 16], f32)
            nc.vector.tensor_scalar(gidx_f[:], gsb[:, :, 0], float(ZROW),
                                    float(ZROW), op0=ALU.add, op1=ALU.min)
            nc.vector.tensor_scalar_max(gidx_f[:], gidx_f[:], 0.0)
            sidx_f = ipool.tile([128, S // 16], f32)
            nc.vector.tensor_scalar(sidx_f[:], gsb[:, :, 1], float(DUMMY),
                                    float(DUMMY), op0=ALU.add, op1=ALU.min)
            nc.vector.tensor_scalar_max(sidx_f[:], sidx_f[:], 0.0)

            gidx16 = ipool.tile([128, S // 16], i16)
            sidx16 = ipool.tile([128, S // 16], i16)
            nc.vector.tensor_copy(gidx16[:], gidx_f[:])
            nc.vector.tensor_copy(sidx16[:], sidx_f[:])

            # ---------------- shared expert (bf16) ----------------
            h_sh = apool.tile([128, KT, TLOC], bf16)
            for ho in range(KT):
                mo = slice(ho * 128, (ho + 1) * 128)
                for th in range(2):
                    ts = slice(th * 512, (th + 1) * 512)
                    ps = ps_l1.tile([128, 512], f32, tag="l1")
                    for k in range(KT):
                        nc.tensor.matmul(ps[:], wfc_sb[:, k, mo],
                                         x16[:, k, ts],
                                         start=(k == 0), stop=(k == KT - 1))
                    rl = yshpool.tile([128, 512], f32, tag="rl")
                    nc.vector.tensor_scalar_max(rl[:], ps[:], 0.0)
                    nc.scalar.activation(h_sh[:, ho, ts], rl[:], AF.Square)
            for tcb in range(TB):
                tcs = slice(tcb * 128, (tcb + 1) * 128)
                ysh_t = yshpool.tile([128, C], f32, tag="ysh")
                for half in range(2):
                    hs = slice(half * 384, (half + 1) * 384)
                    psx = ps_l2.tile([128, 384], f32, tag=f"l2{half}")
                    for hk in range(KT):
                        nc.tensor.matmul(psx[:], h_sh[:, hk, tcs],
                                         wproj_sb[:, hk, hs],
                                         start=(hk == 0), stop=(hk == KT - 1))
                    nc.scalar.activation(ysh_t[:, hs], psx[:], AF.Copy)
                nc.sync.dma_start(o_out[tcb * 128:(tcb + 1) * 128, :], ysh_t[:])
            zrow32 = ipool.tile([128, C], f32)
            nc.vector.memset(zrow32[:], 0.0)
            nc.sync.dma_start(o_out[DUMMY:DUMMY + 128, :], zrow32[:])

            # ---------------- routed experts ----------------
            def emit_gather(e):
                xg = gpool.tile([128, KT, CAP], bf16, tag="xg")
                nc.gpsimd.dma_gather(xg[:], xs_dram[:],
                                     gidx16[:, ECOLS * e:ECOLS * (e + 1)],
                                     CAP, CAP, C, transpose=True)
                return xg

            def emit_weights(e):
                w1sb = wpool.tile([128, KT, C], bf16, tag="w1")
                w2sb = wpool.tile([128, KT, C], bf16, tag="w2")
                w1_src = w1[e].rearrange("(k p) m -> p k m", p=128)
                w2_src = w2[e].rearrange("(k p) m -> p k m", p=128)
                for k in range(KT):
                    nc.sync.dma_start(w1sb[:, k, :], w1_src[:, k, :])
                    nc.sync.dma_start(w2sb[:, k, :], w2_src[:, k, :])
                return w1sb, w2sb

            def emit_l1(e, xg, w1sb):
                hsq = hpool.tile([128, KT, CAP], bf16, tag="hsq")
                for ho in range(KT):
                    mo = slice(ho * 128, (ho + 1) * 128)
                    ps = ps_l1.tile([128, 512], f32, tag="l1")
                    for k in range(KT):
                        nc.tensor.matmul(ps[:, 0:CAP], w1sb[:, k, mo],
                                         xg[:, k, :],
                                         start=(k == 0), stop=(k == KT - 1))
                    rl = yshpool.tile([128, 512], f32, tag="rl")
                    nc.vector.tensor_scalar_max(rl[:, 0:CAP], ps[:, 0:CAP], 0.0)
                    nc.scalar.activation(hsq[:, ho, :], rl[:, 0:CAP], AF.Square)
                return hsq

            def emit_l2(e, hsq, w2sb):
                ysb = ypool.tile([128, EB, C], f32, tag="ysb")
                for sc in range(EB):
                    scs = slice(sc * 128, (sc + 1) * 128)
                    for half in range(2):
                        hs = slice(half * 384, (half + 1) * 384)
                        psx = ps_l2.tile([128, 384], f32, tag=f"l2{half}")
                        for hk in range(KT):
                            nc.tensor.matmul(psx[:], hsq[:, hk, scs],
                                             w2sb[:, hk, hs],
                                             start=(hk == 0),
                                             stop=(hk == KT - 1))
                        nc.scalar.activation(ysb[:, sc, hs], psx[:], AF.Copy)
                nc.gpsimd.dma_scatter_add(o_out[:], ysb[:],
                                          sidx16[:, ECOLS * e:ECOLS * (e + 1)],
                                          CAP, CAP, C)

            # software pipeline: L1(e+1) is emitted before L2(e) so the PE
            # covers the hsq relu/square latency and the next gather/weights
            xg0 = emit_gather(0)
            w1sb0, w2sb0 = emit_weights(0)
            xg1 = emit_gather(1)
            hsq_prev = emit_l1(0, xg0, w1sb0)
            w2_prev = w2sb0
            xg_cur = xg1
            for e in range(1, E + 1):
                if e < E:
                    w1sb, w2sb = emit_weights(e)
                    xg_next = emit_gather(e + 1) if e + 1 < E else None
                    hsq = emit_l1(e, xg_cur, w1sb)
                emit_l2(e - 1, hsq_prev, w2_prev)
                if e < E:
                    hsq_prev, w2_prev, xg_cur = hsq, w2sb, xg_next
    nc.compile()
    return nc


_NC_CACHE = None


def _get_nc():
    global _NC_CACHE
    if _NC_CACHE is None:
        _NC_CACHE = _build()
    return _NC_CACHE


def make_in_maps(x, w_fc_sh, w_proj_sh, w1, w2, router_w, balance_bias=None):
    x = np.ascontiguousarray(np.asarray(x, np.float32))
    bfl = ml_dtypes.bfloat16
    w1b = np.ascontiguousarray(np.asarray(w1, np.float32).astype(bfl))
    w2b = np.ascontiguousarray(np.asarray(w2, np.float32).astype(bfl))
    wfcb = np.ascontiguousarray(np.asarray(w_fc_sh, np.float32).astype(bfl))
    wprojb = np.ascontiguousarray(np.asarray(w_proj_sh, np.float32).astype(bfl))
    rwT = np.ascontiguousarray(np.asarray(router_w, np.float32).T)
    trid = np.triu(np.ones((128, 128), np.float32), 1)
    e384row = (np.tile(np.arange(E, dtype=np.float32), TB)
               * CAP).reshape(1, TB * E)
    if balance_bias is None:
        balance_bias = np.zeros(E, np.float32)
    brow = np.tile(np.asarray(balance_bias, np.float32).reshape(E), TB)
    brow = np.ascontiguousarray(brow.reshape(1, TB * E))
    p_col = np.arange(128, dtype=np.float32).reshape(128, 1)
    b_idx = np.arange(2 * TB)
    n_base = (b_idx % TB) * 128                      # token block base
    k_of_b = b_idx // TB
    cbase = np.zeros((128, 2 * TB, 2), np.float32)
    cbase[:, :, 0] = p_col + n_base + k_of_b * TLOC - 2 * TLOC
    cbase[:, :, 1] = p_col + n_base - TLOC

    xf = x.reshape(N_TOK, C)
    in_maps = []
    for i in range(N_CORES):
        xs = xf[i * TLOC:(i + 1) * TLOC]
        xT = np.ascontiguousarray(xs.T)
        in_maps.append({
            "x_T": xT,
            "x_T16": np.ascontiguousarray(xT.astype(bfl)),
            "x_tm": np.ascontiguousarray(xs),
            "rwT": rwT, "w1": w1b, "w2": w2b,
            "wfc": wfcb, "wproj": wprojb,
            "trid": trid, "e384row": e384row, "cbase": cbase, "brow": brow,
        })
    return in_maps


def kernel(x, w_fc_sh, w_proj_sh, w1, w2, router_w, balance_bias):
    nc = _get_nc()
    in_maps = make_in_maps(x, w_fc_sh, w_proj_sh, w1, w2, router_w,
                           balance_bias)
    res = run_bass_kernel_spmd(nc, in_maps, list(range(N_CORES)))
    shards = [np.asarray(res.results[i]["o_out"])[:TLOC]
              for i in range(N_CORES)]
    out = np.concatenate(shards, axis=0).reshape(B, T, C).astype(np.float32)
    kernel._last_results = res
    return out
